# revision 42
# baseline (speedup 1.0000x reference)
"""BinaryConv2d Trainium2 kernel (8-core batch-parallel, image-pair PE
quadrant packing).

Images are processed in PAIRS: PE row group 0 (SBUF partitions 0:64) holds
image A's channels, row group 1 holds image B's, and the two PE column
groups compute the top/bottom image halves.  Each 3x3 position is 4
concurrent 64x64 matmuls (tile_position (0,0)/(0,64)/(64,0)/(64,64)) into
two PSUM tiles (psumA = imgA top|bottom on partitions 0:64|64:128, psumB =
imgB).  Because x is declared [n_img*64, h, w], the (image,channel) axis is
one uniform-stride 128-partition dim, so every input load is a single
128-partition SWDGE cast-DMA (f32 HBM -> bf16 SBUF) engaging all 16 SDMA
engines, with halo rows folded into the chunk ranges.  Output accumulates
in SBUF band buffers (bf16, partitions = half*64+cout) and stores one
128-partition DMA per 20-row band (these run ~26 GB/s/engine vs ~16 for
64-partition stores); the host unscrambles [n,2,64,80,160] ->
[n,64,160,160] and upcasts (bf16 rounding ~0.4% << 2e-2 tolerance).

Under 8-core SPMD the SDMA engines are the contended resource (~11-23 GB/s
per engine per stream side), so the f32->bf16 cast halves the input's SBUF
port cost.  Loads are software-pipelined one pair ahead; pair 0's
tile-0-critical rows go via HWDGE f32 with a Q7 gate holding back the SWDGE
bulk until they land (packet round-robin has no priority).
"""
import sys
import numpy as np
from contextlib import ExitStack

sys.path.insert(0, "/root/.axon_site/_ro/trn_rl_repo")
sys.path.insert(0, "/opt/trn_rl_repo")

import ml_dtypes
import concourse.bass as bass
import concourse.bacc as bacc
import concourse.mybir as mybir
import concourse.tile as tile
from concourse.bass_utils import run_bass_kernel_spmd

F32 = mybir.dt.float32
BF16 = mybir.dt.bfloat16

N_CORES = 8
B, CIN, COUT, KS = 32, 64, 64, 3
H = W = 160
B_CORE = B // N_CORES
HF = H // 2            # rows per image half (80)
SH = HF + 2            # slab rows per half (1 halo/pad row each side)
PW = W + 2
RPT = 3                # output rows per PSUM tile


def build_nc(n_img=B_CORE, h=H, w=W):
    hf = h // 2
    sh = hf + 2
    pw = w + 2
    qh = hf // 4  # 20-row store bands
    n_pairs = n_img // 2
    nc = bacc.Bacc("TRN2", target_bir_lowering=False, debug=False, num_devices=N_CORES)
    x_in = nc.declare_dram_parameter("x", [n_img * CIN, h, w], F32, isOutput=False)
    wsgn_in = nc.declare_dram_parameter("wsgn", [128, 9 * 64], BF16, isOutput=False)
    scale_in = nc.declare_dram_parameter("scale", [128, 1], F32, isOutput=False)
    # out[img, half*64+cout, r, w] covers output row half*80+r
    out_ext = nc.declare_dram_parameter("out", [n_img, 128, hf, w], BF16, isOutput=True)

    n_tiles = (qh + RPT - 1) // RPT  # 6 full + 1 leftover (R=2) per band

    with tile.TileContext(nc) as tc, ExitStack() as ctx:
        wpool = ctx.enter_context(tc.tile_pool(name="wpool", bufs=1))
        spool = ctx.enter_context(tc.tile_pool(name="spool", bufs=1))
        xpool = ctx.enter_context(tc.tile_pool(name="xpool", bufs=2))
        ppool = ctx.enter_context(tc.tile_pool(name="ppool", bufs=3, space="PSUM"))
        opool = ctx.enter_context(tc.tile_pool(name="opool", bufs=2))

        wt2 = wpool.tile([128, 9 * 64], BF16, name="wt2")
        nc.sync.dma_start(wt2[:], wsgn_in[:])
        sc2 = wpool.tile([128, 1], F32, name="sc2")
        gdum = wpool.tile([128, 1], BF16, name="gdum")
        gdum2 = wpool.tile([128, 1], BF16, name="gdum2")

        # Per-pair slab sets.  slabA row s = x row s-1 of the TOP half
        # (s=0 is the zero top pad); slabB row s = x row hf-1+s of the
        # BOTTOM half (s=sh-1 is the zero bottom pad).  Partitions 0:64 =
        # image A's 64 channels, 64:128 = image B's.
        slabs = []
        for i in range(2):
            slabA = spool.tile([128, sh * pw], BF16, name=f"slabA{i}", tag=f"slabA{i}")
            slabB = spool.tile([128, sh * pw], BF16, name=f"slabB{i}", tag=f"slabB{i}")
            for slab in (slabA, slabB):
                nc.vector.memset(slab[:, 0 : (sh - 1) * pw + pw : pw], 0.0)
                nc.vector.memset(slab[:, pw - 1 : sh * pw : pw], 0.0)
            sA3 = slabA.rearrange("p (r c) -> p r c", c=pw)
            sB3 = slabB.rearrange("p (r c) -> p r c", c=pw)
            nc.vector.memset(sA3[:, 0, :], 0.0)       # image top pad
            nc.vector.memset(sB3[:, sh - 1, :], 0.0)  # image bottom pad
            slabs.append((slabA, slabB))

        def s3(pair):
            slabA, slabB = slabs[pair % 2]
            return (
                slabA.rearrange("p (r c) -> p r c", c=pw),
                slabB.rearrange("p (r c) -> p r c", c=pw),
            )

        # slab-row chunk lists (A: rows 1..sh-1 <- x rows 0..hf, B: rows
        # 0..sh-2 <- x rows hf-1..h-1).  Pair 0 is finely chunked so the PE
        # ramps while loads stream; pair 1 is coarse (fewer SWDGE DMAs).
        a_chunks0 = [(1, 6), (6, 19), (19, 32), (32, 45), (45, 58), (58, 70), (70, 82)]
        b_chunks0 = [(0, 5), (5, 18), (18, 31), (31, 44), (44, 57), (57, 69), (69, 81)]
        a_chunks1 = [(1, 28), (28, 55), (55, 82)]
        b_chunks1 = [(0, 27), (27, 54), (54, 81)]

        def emit_loads(pair):
            first = pair == 0
            c0 = pair * 2 * CIN
            sA3, sB3 = s3(pair)
            ach = a_chunks0 if first else a_chunks1
            bch = b_chunks0 if first else b_chunks1
            xb = None
            for ci, ((a0, a1), (b0, b1)) in enumerate(zip(ach, bch)):
                crit = first and ci == 0
                # A and B get separate staging tags so a pair's first A/B
                # chunks never wait on each other's sign (buffer WAR)
                na = a1 - a0
                atag = "xc5" if crit else f"xa{na}"
                xa = xpool.tile([128, na * w], BF16, name="xa", tag=atag)
                xa3 = xa.rearrange("p (r c) -> p r c", c=w)
                nc.gpsimd.dma_start(xa[:], x_in[c0 : c0 + 128, a0 - 1 : a1 - 1, :])
                # the chunk feeding each pair's first tiles gets its sign
                # split in two, so those tiles unblock after a short ACT op
                # instead of the full-chunk one
                amid = a0 + 6 if ci == (1 if first else 0) else a1
                for lo, hi in ((a0, min(amid, a1)), (min(amid, a1), a1)):
                    if hi > lo:
                        nc.scalar.sign(
                            sA3[:, lo:hi, 1 : 1 + w], xa3[:, lo - a0 : hi - a0, :]
                        )

                nb = b1 - b0
                btag = "xc5" if crit else f"xb{nb}"
                xb = xpool.tile([128, nb * w], BF16, name="xb", tag=btag)
                xb3 = xb.rearrange("p (r c) -> p r c", c=w)
                nc.gpsimd.dma_start(xb[:], x_in[c0 : c0 + 128, hf - 1 + b0 : hf - 1 + b1, :])
                bmid = b0 + 6 if ci == (1 if first else 0) else b1
                for lo, hi in ((b0, min(bmid, b1)), (min(bmid, b1), b1)):
                    if hi > lo:
                        nc.scalar.sign(
                            sB3[:, lo:hi, 1 : 1 + w], xb3[:, lo - b0 : hi - b0, :]
                        )

                if crit:
                    # Q7 gate: stall SWDGE emission (the bulk prefetch)
                    # until pair 0's first critical chunk has landed, so
                    # tile 0 starts early.
                    nc.gpsimd.tensor_scalar_add(gdum[0:64, :], xa[0:64, 0:1], 0.0)
                    nc.sync.dma_start(sc2[:], scale_in[:])
            return xb

        def emit_compute_stores(pair):
            sA3, sB3 = s3(pair)
            imgA, imgB = 2 * pair, 2 * pair + 1
            last = pair == n_pairs - 1

            for band in range(4):
                r0 = band * qh
                obufA = opool.tile([128, qh * w], BF16, name="obA", tag="obA")
                obufB = opool.tile([128, qh * w], BF16, name="obB", tag="obB")
                for t in range(n_tiles):
                    h0 = t * RPT
                    R = min(RPT, qh - h0)
                    s0 = r0 + h0  # slab row base for this tile
                    psumA = ppool.tile([128, R * w], F32, name="psumA", tag="psumA")
                    psumB = ppool.tile([128, R * w], F32, name="psumB", tag="psumB")
                    for kh in range(KS):
                        for kw in range(KS):
                            pos = kh * KS + kw
                            st, sp = (pos == 0), (pos == 8)
                            wA = wt2[0:64, pos * 64 : (pos + 1) * 64]
                            wB = wt2[64:128, pos * 64 : (pos + 1) * 64]
                            rA = sA3[:, s0 + kh : s0 + kh + R, kw : kw + w]
                            rB = sB3[:, s0 + kh : s0 + kh + R, kw : kw + w]
                            nc.tensor.matmul(
                                psumA[0:64, :], wA, rA[0:64],
                                start=st, stop=sp, tile_position=(0, 0),
                            )
                            nc.tensor.matmul(
                                psumA[64:128, :], wA, rB[0:64],
                                start=st, stop=sp, tile_position=(0, 64),
                            )
                            nc.tensor.matmul(
                                psumB[0:64, :], wB, rA[64:128],
                                start=st, stop=sp, tile_position=(64, 0),
                            )
                            nc.tensor.matmul(
                                psumB[64:128, :], wB, rB[64:128],
                                start=st, stop=sp, tile_position=(64, 64),
                            )
                    # scale + downcast into the band accumulators (DVE)
                    nc.vector.tensor_scalar_mul(
                        obufA[:, h0 * w : (h0 + R) * w], psumA[:], sc2[:]
                    )
                    nc.vector.tensor_scalar_mul(
                        obufB[:, h0 * w : (h0 + R) * w], psumB[:], sc2[:]
                    )
                    # last band of the last pair: flush finished rows
                    # mid-compute so only a small tail DMA remains
                    if last and band == 3 and t == 3:
                        hi = h0 + R
                        for im, ob in ((imgA, obufA), (imgB, obufB)):
                            nc.sync.dma_start(
                                out_ext[im, :, r0 : r0 + hi, :], ob[:, 0 : hi * w]
                            )

                if last and band == 3:
                    m = 12 * w  # band rows 0:12 already stored
                    for im, ob in ((imgA, obufA), (imgB, obufB)):
                        nc.sync.dma_start(
                            out_ext[im, :, r0 + 12 : r0 + qh, :], ob[:, m:]
                        )
                else:
                    nc.sync.dma_start(out_ext[imgA, :, r0 : r0 + qh, :], obufA[:])
                    nc.sync.dma_start(out_ext[imgB, :, r0 : r0 + qh, :], obufB[:])

        emit_loads(0)
        for pair in range(n_pairs):
            if pair + 1 < n_pairs:
                emit_loads(pair + 1)
            emit_compute_stores(pair)
    nc.finalize()
    return nc


_NC_CACHE = {}


def _get_nc():
    if "nc" not in _NC_CACHE:
        _NC_CACHE["nc"] = build_nc()
    return _NC_CACHE["nc"]


def _prep_weights(w):
    wc = np.clip(np.asarray(w, dtype=np.float32), -1.0, 1.0)
    scale = np.abs(wc).mean(axis=(1, 2, 3)).astype(np.float32).reshape(64, 1)
    s = np.sign(wc).astype(np.float32)  # [co, ci, kh, kw]
    wsgn = np.ascontiguousarray(
        np.transpose(s, (1, 2, 3, 0)).reshape(64, 9 * 64)
    )
    wsgn2 = np.concatenate([wsgn, wsgn], axis=0).astype(ml_dtypes.bfloat16)
    scale2 = np.concatenate([scale, scale], axis=0)
    return wsgn2, scale2


def kernel(x, w, _trace=False):
    x = np.ascontiguousarray(np.asarray(x, dtype=np.float32))
    wsgn2, scale2 = _prep_weights(w)
    nc = _get_nc()
    in_maps = [
        {
            "x": x[i * B_CORE : (i + 1) * B_CORE].reshape(B_CORE * CIN, H, W),
            "wsgn": wsgn2,
            "scale": scale2,
        }
        for i in range(N_CORES)
    ]
    # The axon-proxied execution occasionally faults with a transient
    # NRT_EXEC_UNIT_UNRECOVERABLE; a retry on a fresh session recovers.
    last_err = None
    for attempt in range(3):
        try:
            res = run_bass_kernel_spmd(nc, in_maps, list(range(N_CORES)), trace=_trace)
            break
        except Exception as e:  # noqa: BLE001
            last_err = e
            import time as _time
            _time.sleep(3.0)
    else:
        raise last_err
    parts = []
    for i in range(N_CORES):
        r = np.asarray(res.results[i]["out"])  # [B_CORE, 128, 80, 160] bf16
        r = r.reshape(B_CORE, 2, COUT, HF, W).transpose(0, 2, 1, 3, 4)
        parts.append(r.reshape(B_CORE, COUT, H, W).astype(np.float32))
    out = np.concatenate(parts, axis=0)
    if _trace:
        return out, res
    return out
